# revision 18
# baseline (speedup 1.0000x reference)
"""Multi-level ROI Align (FPN pooler, 4 levels summed) on 8 Trainium2 cores.

Strategy: shard ROIs across cores (core k: batch k//4, 128 ROIs). Gather
indices and bilinear weights are computed on host from `boxes`; the device
kernel does HBM pixel gathers (bf16) + weighted scatter-reduction into 7x7
bins via PSUM-accumulating bf16 matmuls.

Per ROI, per level:  out[bin, c] = sum_j W[j, bin] * G[j, c]
where G rows are gathered bf16 pixel vectors (C=256) and W is either
fixed_pattern * per-partition scalar built on DVE (L0/L1, one-hot j->bin)
or host-baked dense bf16 lhsT (L2/L3 region mode).

L0 uses 3-px elements addressed at even-pixel granularity (idx = flat//2)
to fit the int16 index range (200*200 = 40000 > 32767).

Output is accumulated in fp32 PSUM, evacuated as bf16 [49, C] per ROI and
DMA'd straight to HBM; the host does the final [49,C] -> [C,7,7] transpose.
"""
import sys
import numpy as np
import ml_dtypes

sys.path.insert(0, '/opt/trn_rl_repo')

BF16 = ml_dtypes.bfloat16

POOLED = 7
SAMP = 2
NBIN = 49
C = 256
IMG = 800.0

# per level: H, W, scale, mode ('tri' 3px elems idx=flat//2 | 'px' 1px | 'reg' region px)
L0 = dict(H=200, W=200, scale=0.25, mode='tri', NJ=400, REAL=392, NCH=4)
L1 = dict(H=100, W=100, scale=0.125, mode='pair', NJ=400, REAL=392, NCH=4)
L2 = dict(H=50, W=50, scale=0.0625, mode='reg', NJ=384, REAL=324, NCH=3, WREG=18)
L3 = dict(H=25, W=25, scale=0.03125, mode='reg', NJ=128, REAL=100, NCH=1, WREG=10)
LEVELS = [L0, L1, L2, L3]

NROI_CORE = 128     # ROIs per core
NGRP = 64           # groups of 2 ROIs
GRP = 2
BLK2 = 2            # ROIs per L2 gather tile (per-group)
BLK3 = 2            # ROIs per L3 gather tile (per-group)

# padded flat pixel counts of the feature buffers
F0_ROWS = 40004     # covers 3-px elem overrun
F1_ROWS = 10002    # covers 3-px elem overrun
F2_ROWS = 3400      # covers region overrun (y,x up to 66)
F3_ROWS = 900       # covers region overrun (y,x up to 33)

# const bf16 column layout (per partition)
PAT0_OFF = 0                                  # [12, 49] pattern expanded per slot
PAT1_OFF = PAT0_OFF + 12 * NBIN               # [8, 49]
WCOL0_OFF = PAT1_OFF + 8 * NBIN               # [128 roi * 12]
WCOL1_OFF = WCOL0_OFF + NROI_CORE * 12        # [128 roi * 8]
CST_COLS = WCOL1_OFF + NROI_CORE * 8

# idx int16 column layout (per partition); all calls are per-ROI,
# trailing -1 entries are self-trimmed by the gather ucode (no DMA traffic).
# L2+L3 share one call: zones [0:384)=L2 (idx-0 padded), [384:512)=L3
# (idx offset by F2_ROWS into the concatenated f23 buffer, -1 tail).
IC0 = L0['NJ'] // 16                          # 25 cols per ROI
IC1 = L1['NJ'] // 16                          # 25
NJ23 = L2['NJ'] + L3['NJ']                    # 512
IC23 = NJ23 // 16                             # 32
IDX0_OFF = 0
IDX1_OFF = IDX0_OFF + NROI_CORE * IC0
IDX23_OFF = IDX1_OFF + NROI_CORE * IC1
IDX_COLS = IDX23_OFF + NROI_CORE * IC23

_MODULE_CACHE = {}


def _sample_meta(boxes_b, H, W, scale):
    """Per-ROI sample geometry in fp32, matching reference op order.
    boxes_b: [N, 4] fp32. Returns dict of [N,7,2] arrays."""
    f = np.float32
    b = boxes_b.astype(np.float32)
    x1 = b[:, 0] * f(scale)
    y1 = b[:, 1] * f(scale)
    x2 = b[:, 2] * f(scale)
    y2 = b[:, 3] * f(scale)
    rw = np.maximum(x2 - x1, f(1.0))
    rh = np.maximum(y2 - y1, f(1.0))
    bw = rw / f(POOLED)
    bh = rh / f(POOLED)
    g = (np.arange(POOLED, dtype=np.float32)[:, None]
         + (np.arange(SAMP, dtype=np.float32)[None, :] + f(0.5)) / f(SAMP))
    y = y1[:, None, None] + g[None] * bh[:, None, None]   # [N,7,2]
    x = x1[:, None, None] + g[None] * bw[:, None, None]
    masky = ((y >= f(-1.0)) & (y <= f(H))).astype(np.float32)
    maskx = ((x >= f(-1.0)) & (x <= f(W))).astype(np.float32)
    yc = np.clip(y, f(0.0), f(H - 1))
    xc = np.clip(x, f(0.0), f(W - 1))
    yl = np.floor(yc).astype(np.int64)
    xl = np.floor(xc).astype(np.int64)
    yh = np.minimum(yl + 1, H - 1)
    xh = np.minimum(xl + 1, W - 1)
    ly = (yc - yl.astype(np.float32)).astype(np.float32)
    lx = (xc - xl.astype(np.float32)).astype(np.float32)
    hy = (f(1.0) - ly).astype(np.float32)
    hx = (f(1.0) - lx).astype(np.float32)
    return dict(yl=yl, yh=yh, xl=xl, xh=xh, ly=ly, lx=lx, hy=hy, hx=hx,
                masky=masky, maskx=maskx, x=x, y=y)


def _build_tri(meta, lv):
    """L0: j = (rs, py, sy, px, sx) -> 392 3-px elems, 3 slot weights.
    Returns idx [N, NJ] int64, w [N, NJ, 3] fp32."""
    N = meta['yl'].shape[0]
    W = lv['W']
    NJ, REAL = lv['NJ'], lv['REAL']
    rows = np.stack([meta['yl'], meta['yh']], axis=1)          # [N,2,7,2] (rs)
    wys = np.stack([meta['hy'], meta['ly']], axis=1)           # [N,2,7,2]
    m = (meta['masky'][:, :, :, None, None] * meta['maskx'][:, None, None, :, :])  # [N,7,2,7,2]
    row = np.broadcast_to(rows[:, :, :, :, None, None], (N, 2, 7, 2, 7, 2))
    wy = np.broadcast_to(wys[:, :, :, :, None, None], (N, 2, 7, 2, 7, 2)).astype(np.float32)
    xl = np.broadcast_to(meta['xl'][:, None, None, None, :, :], (N, 2, 7, 2, 7, 2))
    hx = np.broadcast_to(meta['hx'][:, None, None, None, :, :], (N, 2, 7, 2, 7, 2)).astype(np.float32)
    lx = np.broadcast_to(meta['lx'][:, None, None, None, :, :], (N, 2, 7, 2, 7, 2)).astype(np.float32)
    mm = np.broadcast_to(m[:, None], (N, 2, 7, 2, 7, 2)).astype(np.float32)
    flat = row * W + xl
    idx = (flat >> 1).reshape(N, REAL)
    r = (flat & 1).astype(np.float32).reshape(N, REAL)
    wl = (wy * hx * mm * np.float32(0.25)).reshape(N, REAL)
    wh = (wy * lx * mm * np.float32(0.25)).reshape(N, REAL)
    w = np.zeros((N, NJ, 3), np.float32)
    w[:, :REAL, 0] = wl * (1 - r)
    w[:, :REAL, 1] = wl * r + wh * (1 - r)
    w[:, :REAL, 2] = wh * r
    idx_full = np.full((N, NJ), -1, np.int64)
    idx_full[:, :REAL] = idx
    return idx_full, w


def _build_pair(meta, lv):
    """2px overlapping elems at 1px step: elem j=(rs,py,sy,px,sx) starts at
    (row, xl): slot0=xl (wy*hx), slot1=xl+1 (wy*lx). idx = row*W + xl."""
    N = meta['yl'].shape[0]
    W = lv['W']
    NJ, REAL = lv['NJ'], lv['REAL']
    rows = np.stack([meta['yl'], meta['yh']], axis=1)          # [N,2,7,2]
    wys = np.stack([meta['hy'], meta['ly']], axis=1)
    m = (meta['masky'][:, :, :, None, None] * meta['maskx'][:, None, None, :, :])
    row = np.broadcast_to(rows[:, :, :, :, None, None], (N, 2, 7, 2, 7, 2))
    wy = np.broadcast_to(wys[:, :, :, :, None, None], (N, 2, 7, 2, 7, 2)).astype(np.float32)
    xl = np.broadcast_to(meta['xl'][:, None, None, None, :, :], (N, 2, 7, 2, 7, 2))
    hx = np.broadcast_to(meta['hx'][:, None, None, None, :, :], (N, 2, 7, 2, 7, 2)).astype(np.float32)
    lx = np.broadcast_to(meta['lx'][:, None, None, None, :, :], (N, 2, 7, 2, 7, 2)).astype(np.float32)
    mm = np.broadcast_to(m[:, None], (N, 2, 7, 2, 7, 2)).astype(np.float32)
    idx = (row * W + xl).reshape(N, REAL)
    w = np.zeros((N, NJ, 2), np.float32)
    w[:, :REAL, 0] = (wy * hx * mm * np.float32(0.25)).reshape(N, REAL)
    w[:, :REAL, 1] = (wy * lx * mm * np.float32(0.25)).reshape(N, REAL)
    idx_full = np.full((N, NJ), -1, np.int64)
    idx_full[:, :REAL] = idx
    return idx_full, w


def _build_px(meta, lv):
    """L1: j = (rs, cs, py, sy, px, sx) -> 784 1-px corner gathers.
    Returns idx [N, NJ] int64, w [N, NJ] fp32."""
    N = meta['yl'].shape[0]
    W = lv['W']
    NJ, REAL = lv['NJ'], lv['REAL']
    rows = np.stack([meta['yl'], meta['yh']], axis=1)   # [N,2(rs),7,2]
    wys = np.stack([meta['hy'], meta['ly']], axis=1)
    cols = np.stack([meta['xl'], meta['xh']], axis=1)   # [N,2(cs),7,2]
    wxs = np.stack([meta['hx'], meta['lx']], axis=1)
    m = (meta['masky'][:, :, :, None, None] * meta['maskx'][:, None, None, :, :])
    row = np.broadcast_to(rows[:, :, None, :, :, None, None], (N, 2, 2, 7, 2, 7, 2))
    wy = np.broadcast_to(wys[:, :, None, :, :, None, None], (N, 2, 2, 7, 2, 7, 2)).astype(np.float32)
    col = np.broadcast_to(cols[:, None, :, None, None, :, :], (N, 2, 2, 7, 2, 7, 2))
    wx = np.broadcast_to(wxs[:, None, :, None, None, :, :], (N, 2, 2, 7, 2, 7, 2)).astype(np.float32)
    mm = np.broadcast_to(m[:, None, None], (N, 2, 2, 7, 2, 7, 2)).astype(np.float32)
    idx = (row * W + col).reshape(N, REAL)
    w = (wy * wx * mm * np.float32(0.25)).reshape(N, REAL)
    idx_full = np.full((N, NJ), -1, np.int64)
    w_full = np.zeros((N, NJ), np.float32)
    idx_full[:, :REAL] = idx
    w_full[:, :REAL] = w
    return idx_full, w_full


def _build_reg(meta, lv, pad_value=0):
    """L2/L3: bounding-region pixels + separable host-baked weights.
    Returns idx [N, NJ] int64, lhsT [N, NJ, 49] fp32.
    pad_value=-1 trims pad descriptors (only safe when NJ is one 128-chunk)."""
    N = meta['yl'].shape[0]
    H, W, WREG = lv['H'], lv['W'], lv['WREG']
    NJ, REAL = lv['NJ'], lv['REAL']
    f = np.float32
    y_base = np.floor(np.clip(meta['y'].reshape(N, -1).min(1), 0.0, H - 1)).astype(np.int64)
    x_base = np.floor(np.clip(meta['x'].reshape(N, -1).min(1), 0.0, W - 1)).astype(np.int64)
    WY = np.zeros((N, WREG, POOLED), np.float32)
    WX = np.zeros((N, WREG, POOLED), np.float32)
    ridx = np.arange(N)[:, None, None]
    pidx = np.broadcast_to(np.arange(POOLED)[None, :, None], (N, POOLED, SAMP))
    np.add.at(WY, (ridx, meta['yl'] - y_base[:, None, None], pidx),
              (f(0.5) * meta['hy'] * meta['masky']).astype(np.float32))
    np.add.at(WY, (ridx, meta['yh'] - y_base[:, None, None], pidx),
              (f(0.5) * meta['ly'] * meta['masky']).astype(np.float32))
    np.add.at(WX, (ridx, meta['xl'] - x_base[:, None, None], pidx),
              (f(0.5) * meta['hx'] * meta['maskx']).astype(np.float32))
    np.add.at(WX, (ridx, meta['xh'] - x_base[:, None, None], pidx),
              (f(0.5) * meta['lx'] * meta['maskx']).astype(np.float32))
    # variable spans: last nonzero WY/WX row per ROI -> sy, sx
    ny = (WY.reshape(N, WREG, -1) != 0).any(-1)    # [N, WREG]
    nx = (WX.reshape(N, WREG, -1) != 0).any(-1)
    sy = np.maximum(WREG - np.argmax(ny[:, ::-1], axis=1), 1)
    sx = np.maximum(WREG - np.argmax(nx[:, ::-1], axis=1), 1)
    lhsT_grid = np.einsum('nap,nbq->nabpq', WY, WX).reshape(N, WREG, WREG, NBIN)
    idx_full = np.full((N, NJ), pad_value, np.int64)
    lhsT_full = np.zeros((N, NJ, NBIN), np.float32)
    for n in range(N):
        yy, xx = int(sy[n]), int(sx[n])
        cnt = yy * xx
        dy = np.arange(yy)
        dx = np.arange(xx)
        idx_full[n, :cnt] = ((y_base[n] + dy[:, None]) * W
                             + x_base[n] + dx[None, :]).reshape(-1)
        lhsT_full[n, :cnt] = lhsT_grid[n, :yy, :xx].reshape(cnt, NBIN)
    return idx_full, lhsT_full


def _pack_idx(jlists):
    """Pack concatenated idx list [..., n] -> [..., 128, n//16]
    int16 wrapped in 16 partitions, replicated 8x."""
    jl = np.asarray(jlists)
    n = jl.shape[-1]
    arr = jl.reshape(*jl.shape[:-1], n // 16, 16)   # [..., col, p]
    arr = np.swapaxes(arr, -1, -2)                  # [..., p(16), col]
    arr = np.broadcast_to(arr[..., None, :, :],
                          (*jl.shape[:-1], 8, 16, n // 16))
    return arr.reshape(*jl.shape[:-1], 128, n // 16).astype(np.int16)


def _bin_pattern(mode, NCH, REAL, nslot):
    """Fixed j->bin one-hot pattern [128, NCH*nslot, 49] (expanded per slot)."""
    NJ = NCH * 128
    j = np.arange(NJ)
    # j = ((((rs*7+py)*2+sy)*7+px)*2+sx)  (same py/px decode for tri & px)
    px = (j // 2) % 7
    py = (j // (2 * 7 * 2)) % 7
    bins = py * 7 + px
    pat = np.zeros((NJ, NBIN), np.float32)
    valid = j < REAL
    pat[np.arange(NJ)[valid], bins[valid]] = 1.0
    pat = pat.reshape(NCH, 128, NBIN).transpose(1, 0, 2)          # [128, NCH, 49]
    pat = np.repeat(pat[:, :, None, :], nslot, axis=2)            # [128, NCH, nslot, 49]
    return pat.reshape(128, NCH * nslot, NBIN)


def _host_prepare(x0, x1, x2, x3, boxes):
    """Build all per-core input tensors. Returns list of 8 dicts."""
    B = boxes.shape[0]
    feats = []
    for arr, lv, rows in ((x0, L0, F0_ROWS), (x1, L1, F1_ROWS),
                          (x2, L2, F2_ROWS), (x3, L3, F3_ROWS)):
        f = np.zeros((B, rows, C), BF16)
        hw = lv['H'] * lv['W']
        f[:, :hw] = np.ascontiguousarray(
            np.transpose(np.asarray(arr, np.float32), (0, 2, 3, 1))).reshape(B, hw, C).astype(BF16)
        feats.append(f)
    f23 = np.concatenate([feats[2], feats[3]], axis=1)   # [B, F2+F3 rows, C]

    per_batch = []
    for b in range(B):
        bb = np.asarray(boxes[b], np.float32)
        m0 = _sample_meta(bb, L0['H'], L0['W'], L0['scale'])
        m1 = _sample_meta(bb, L1['H'], L1['W'], L1['scale'])
        m2 = _sample_meta(bb, L2['H'], L2['W'], L2['scale'])
        m3 = _sample_meta(bb, L3['H'], L3['W'], L3['scale'])
        idx0, w0 = _build_tri(m0, L0)
        idx1, w1 = _build_pair(m1, L1)
        idx2, lt2 = _build_reg(m2, L2, pad_value=0)
        idx3, lt3 = _build_reg(m3, L3, pad_value=-1)
        per_batch.append((idx0, w0, idx1, w1, idx2, lt2, idx3, lt3))

    pat0 = _bin_pattern('tri', L0['NCH'], L0['REAL'], 3)   # [128, 12, 49]
    pat1 = _bin_pattern('pair', L1['NCH'], L1['REAL'], 2)  # [128, 8, 49]

    in_maps = []
    for k in range(8):
        b = k // 4
        s = (k % 4) * NROI_CORE
        idx0, w0, idx1, w1, idx2, lt2, idx3, lt3 = per_batch[b]
        sl = slice(s, s + NROI_CORE)

        cst = np.zeros((128, CST_COLS), BF16)
        cst[:, PAT0_OFF:PAT0_OFF + 12 * NBIN] = pat0.reshape(128, -1).astype(BF16)
        cst[:, PAT1_OFF:PAT1_OFF + 8 * NBIN] = pat1.reshape(128, -1).astype(BF16)
        # wcol0 [128, roi*12]: col roi*12 + c*3 + slot = w0[roi, c*128+p, slot]
        w0p = np.zeros((NROI_CORE, L0['NCH'] * 128, 3), np.float32)
        w0p[:, :L0['NJ']] = w0[sl]
        wc0 = w0p.reshape(NROI_CORE, L0['NCH'], 128, 3)      # [roi,c,p,s]
        cst[:, WCOL0_OFF:WCOL0_OFF + NROI_CORE * 12] = (
            wc0.transpose(2, 0, 1, 3).reshape(128, -1).astype(BF16))
        w1p = np.zeros((NROI_CORE, L1['NCH'] * 128, 2), np.float32)
        w1p[:, :L1['NJ']] = w1[sl]
        wc1 = w1p.reshape(NROI_CORE, L1['NCH'], 128, 2)      # [roi,c,p,s]
        cst[:, WCOL1_OFF:WCOL1_OFF + NROI_CORE * 8] = (
            wc1.transpose(2, 0, 1, 2 + 1).reshape(128, -1).astype(BF16))

        idx3s = idx3[sl].copy()
        idx3s[idx3s >= 0] += F2_ROWS
        idx23 = np.concatenate([idx2[sl], idx3s], axis=1)    # [128roi, 512]
        idxs = np.zeros((128, IDX_COLS), np.int16)
        idxs[:, IDX0_OFF:IDX0_OFF + NROI_CORE * IC0] = _pack_idx(
            idx0[sl]).transpose(1, 0, 2).reshape(128, -1)
        idxs[:, IDX1_OFF:IDX1_OFF + NROI_CORE * IC1] = _pack_idx(
            idx1[sl]).transpose(1, 0, 2).reshape(128, -1)
        idxs[:, IDX23_OFF:IDX23_OFF + NROI_CORE * IC23] = _pack_idx(
            idx23).transpose(1, 0, 2).reshape(128, -1)

        # lhsT k-major bf16: lt2 [roi, NJ(=3*128), 49] -> [roi, 128, 3*49]
        lt2k = np.ascontiguousarray(
            lt2[sl].reshape(NROI_CORE, L2['NCH'], 128, NBIN).transpose(0, 2, 1, 3)
        ).reshape(NROI_CORE, 128, L2['NCH'] * NBIN).astype(BF16)
        lt3k = np.ascontiguousarray(lt3[sl].reshape(NROI_CORE, 128, NBIN)).astype(BF16)

        in_maps.append({
            "f0": feats[0][b], "f1": feats[1][b], "f23": f23[b],
            "cst": cst, "idxs": idxs, "lt2": lt2k, "lt3": lt3k,
        })
    return in_maps


def _build_module():
    from concourse import bacc, tile
    from concourse.bass import mybir
    import concourse.bass as bass_mod

    F32 = mybir.dt.float32
    BF = mybir.dt.bfloat16
    I16 = mybir.dt.int16
    AP = bass_mod.AP

    nc = bacc.Bacc(None, target_bir_lowering=False, num_swdge_queues=4)
    f0 = nc.dram_tensor("f0", [F0_ROWS, C], BF, kind="ExternalInput")
    f1 = nc.dram_tensor("f1", [F1_ROWS, C], BF, kind="ExternalInput")
    f23 = nc.dram_tensor("f23", [F2_ROWS + F3_ROWS, C], BF, kind="ExternalInput")
    cst = nc.dram_tensor("cst", [128, CST_COLS], BF, kind="ExternalInput")
    idxs = nc.dram_tensor("idxs", [128, IDX_COLS], I16, kind="ExternalInput")
    lt2 = nc.dram_tensor("lt2", [NROI_CORE, 128, L2['NCH'] * NBIN], BF, kind="ExternalInput")
    lt3 = nc.dram_tensor("lt3", [NROI_CORE, 128, NBIN], BF, kind="ExternalInput")
    out = nc.dram_tensor("out", [NROI_CORE, NBIN, C], BF, kind="ExternalOutput")

    # overlapping 3-px elem views: stride 2px, width 3px
    f0_view = AP(f0, 0, [[2 * C, F0_ROWS // 2 - 1], [1, 3 * C]])
    f1_view = AP(f1, 0, [[C, F1_ROWS - 1], [1, 2 * C]])

    with tile.TileContext(nc) as tc:
        with (
            tc.tile_pool(name="const", bufs=1) as constp,
            tc.tile_pool(name="g0p", bufs=10) as g0p,
            tc.tile_pool(name="g1p", bufs=10) as g1p,
            tc.tile_pool(name="g2p", bufs=12) as g2p,
            tc.tile_pool(name="ltp", bufs=4) as ltp,
            tc.tile_pool(name="wp", bufs=8) as wp,
            tc.tile_pool(name="accp", bufs=8, space="PSUM") as accp,
            tc.tile_pool(name="evp", bufs=3) as evp,
        ):
            cst_t = constp.tile([128, CST_COLS], BF)
            nc.sync.dma_start(cst_t[:], cst[:])
            idx_t = constp.tile([128, IDX_COLS], I16)
            nc.sync.dma_start(idx_t[:], idxs[:])

            pat0_ap = cst_t[:, PAT0_OFF:PAT0_OFF + 12 * NBIN].rearrange(
                "p (c b) -> p c b", b=NBIN)
            pat1_ap = cst_t[:, PAT1_OFF:PAT1_OFF + 8 * NBIN].rearrange(
                "p (c b) -> p c b", b=NBIN)
            wcol0_ap = cst_t[:, WCOL0_OFF:WCOL0_OFF + NROI_CORE * 12].rearrange(
                "p (r c) -> p r c", c=12)
            wcol1_ap = cst_t[:, WCOL1_OFF:WCOL1_OFF + NROI_CORE * 8].rearrange(
                "p (r c) -> p r c", c=8)

            # zero the rotating gather buffers once: trimmed gather calls leave
            # pad rows unwritten, and the zero-weight protection needs finite
            # data there (never-NaN) on the very first uses.
            for pool, shape, tg, nb in ((g0p, [128, L0['NCH'], 3 * C], "g0", 10),
                                        (g1p, [128, L1['NCH'], 2 * C], "g1", 10),
                                        (g2p, [128, 4, C], "g2", 12)):
                for _ in range(nb):
                    t = pool.tile(shape, BF, tag=tg)
                    nc.vector.memset(t[:], 0)

            gt0s, gt1s, gt23s = {}, {}, {}
            for grp in range(NGRP):
                # per-ROI gathers, all levels, rotating queues, per-ROI tiles
                for r2 in range(GRP):
                    roi = grp * GRP + r2
                    gt0 = g0p.tile([128, L0['NCH'], 3 * C], BF, tag="g0")
                    gt1 = g1p.tile([128, L1['NCH'], 2 * C], BF, tag="g1")
                    gt23 = g2p.tile([128, 4, C], BF, tag="g2")
                    gt0s[roi], gt1s[roi], gt23s[roi] = gt0, gt1, gt23
                    io = IDX0_OFF + roi * IC0
                    nc.gpsimd.dma_gather(
                        gt0[:], f0_view,
                        idx_t[:, io:io + IC0],
                        L0['NJ'], L0['REAL'], 3 * C, elem_step=2 * C,
                        queue_num=roi % 4)
                    io = IDX1_OFF + roi * IC1
                    nc.gpsimd.dma_gather(
                        gt1[:], f1_view,
                        idx_t[:, io:io + IC1],
                        L1['NJ'], L1['REAL'], 2 * C, elem_step=C,
                        queue_num=(roi + 1) % 4)
                    io = IDX23_OFF + roi * IC23
                    nc.gpsimd.dma_gather(
                        gt23[:], f23[:],
                        idx_t[:, io:io + IC23],
                        NJ23, NJ23, C, queue_num=(roi + 2) % 4)
                if grp % 4 == 0:
                    lt2_t = ltp.tile([128, 4 * GRP, L2['NCH'] * NBIN], BF, tag="lt2")
                    nc.sync.dma_start(
                        lt2_t[:],
                        lt2[grp * GRP:(grp + 4) * GRP].rearrange("r p m -> p r m"))
                    lt3_t = ltp.tile([128, 4 * GRP, NBIN], BF, tag="lt3")
                    nc.sync.dma_start(
                        lt3_t[:],
                        lt3[grp * GRP:(grp + 4) * GRP].rearrange("r p m -> p r m"))

                if grp % 2 == 0:
                    ev = evp.tile([NBIN, 2 * GRP, C], BF, tag="ev")
                for r2 in range(GRP):
                    roi = grp * GRP + r2
                    gt0, gt1, gt23 = gt0s.pop(roi), gt1s.pop(roi), gt23s.pop(roi)
                    # build W tiles: pattern * per-partition scalar (broadcast)
                    w0 = wp.tile([128, 12, NBIN], BF, tag="w0")
                    nc.vector.tensor_tensor(
                        w0[:], pat0_ap,
                        wcol0_ap[:, roi, :].unsqueeze(2).to_broadcast([128, 12, NBIN]),
                        mybir.AluOpType.mult)
                    w1 = wp.tile([128, 8, NBIN], BF, tag="w1")
                    nc.vector.tensor_tensor(
                        w1[:], pat1_ap,
                        wcol1_ap[:, roi, :].unsqueeze(2).to_broadcast([128, 8, NBIN]),
                        mybir.AluOpType.mult)

                    acc = accp.tile([NBIN, C], F32)
                    n_mm = 12 + 8 + 3 + 1
                    mi = 0
                    for gt, w_t, lv, nslot in ((gt0, w0, L0, 3), (gt1, w1, L1, 2)):
                        for c in range(lv['NCH']):
                            for sl3 in range(nslot):
                                nc.tensor.matmul(
                                    acc[:], w_t[:, c * nslot + sl3, :],
                                    gt[:, c, sl3 * C:(sl3 + 1) * C],
                                    start=(mi == 0), stop=(mi == n_mm - 1))
                                mi += 1
                    lt2_ap = lt2_t[:, (grp % 4) * GRP + r2, :].rearrange(
                        "p (c b) -> p c b", b=NBIN)
                    for c in range(L2['NCH']):
                        nc.tensor.matmul(
                            acc[:], lt2_ap[:, c, :],
                            gt23[:, c, :],
                            start=(mi == 0), stop=(mi == n_mm - 1))
                        mi += 1
                    nc.tensor.matmul(
                        acc[:], lt3_t[:, (grp % 4) * GRP + r2, :],
                        gt23[:, 3, :],
                        start=(mi == 0), stop=(mi == n_mm - 1))
                    mi += 1

                    nc.scalar.copy(ev[:, (grp % 2) * GRP + r2, :], acc[:])

                if grp % 2 == 1:
                    dst = out[(grp - 1) * GRP:(grp + 1) * GRP].rearrange(
                        "r b c -> b r c")
                    nc.sync.dma_start(dst, ev[:])
    nc.finalize()
    return nc


def kernel(x0, x1, x2, x3, boxes):
    from concourse.bass_utils import run_bass_kernel_spmd
    in_maps = _host_prepare(x0, x1, x2, x3, boxes)
    if 'nc' not in _MODULE_CACHE:
        _MODULE_CACHE['nc'] = _build_module()
    nc = _MODULE_CACHE['nc']
    res = run_bass_kernel_spmd(nc, in_maps, list(range(8)))
    globals()['_LAST_RESULTS'] = res
    outs = [np.asarray(res.results[k]["out"]) for k in range(8)]
    full = np.concatenate(outs, axis=0)            # [1024, 49, 256] bf16
    full = full.astype(np.float32).transpose(0, 2, 1)
    return np.ascontiguousarray(full.reshape(1024, C, POOLED, POOLED))


# revision 19
# speedup vs baseline: 1.1895x; 1.1895x over previous
"""Multi-level ROI Align (FPN pooler, 4 levels summed) on 8 Trainium2 cores.

Strategy: shard ROIs across cores (core k: batch k//4, 128 ROIs). Gather
indices and bilinear weights are computed on host from `boxes`; the device
kernel does HBM pixel gathers (bf16) + weighted scatter-reduction into 7x7
bins via PSUM-accumulating bf16 matmuls.

Per ROI, per level:  out[bin, c] = sum_j W[j, bin] * G[j, c]
where G rows are gathered bf16 pixel vectors (C=256) and W is either
fixed_pattern * per-partition scalar built on DVE (L0/L1, one-hot j->bin)
or host-baked dense bf16 lhsT (L2/L3 region mode).

L0 uses 3-px elements addressed at even-pixel granularity (idx = flat//2)
to fit the int16 index range (200*200 = 40000 > 32767).

Output is accumulated in fp32 PSUM, evacuated as bf16 [49, C] per ROI and
DMA'd straight to HBM; the host does the final [49,C] -> [C,7,7] transpose.
"""
import sys
import numpy as np
import ml_dtypes

sys.path.insert(0, '/opt/trn_rl_repo')

BF16 = ml_dtypes.bfloat16

POOLED = 7
SAMP = 2
NBIN = 49
C = 256
IMG = 800.0

# per level: H, W, scale, mode ('tri' 3px elems idx=flat//2 | 'px' 1px | 'reg' region px)
L0 = dict(H=200, W=200, scale=0.25, mode='tri', NJ=400, REAL=392, NCH=4)
L1 = dict(H=100, W=100, scale=0.125, mode='pair', NJ=400, REAL=392, NCH=4)
L2 = dict(H=50, W=50, scale=0.0625, mode='reg', NJ=384, REAL=324, NCH=3, WREG=18)
L3 = dict(H=25, W=25, scale=0.03125, mode='reg', NJ=128, REAL=100, NCH=1, WREG=10)
LEVELS = [L0, L1, L2, L3]

NROI_CORE = 128     # ROIs per core
NGRP = 64           # groups of 2 ROIs
GRP = 2
BLK2 = 2            # ROIs per L2 gather tile (per-group)
BLK3 = 2            # ROIs per L3 gather tile (per-group)

# padded flat pixel counts of the feature buffers
F0_ROWS = 40004     # covers 3-px elem overrun
F1_ROWS = 10002    # covers 3-px elem overrun
F2_ROWS = 3400      # covers region overrun (y,x up to 66)
F3_ROWS = 900       # covers region overrun (y,x up to 33)

# const bf16 column layout (per partition)
PAT0_OFF = 0                                  # [12, 49] pattern expanded per slot
PAT1_OFF = PAT0_OFF + 12 * NBIN               # [8, 49]
WCOL0_OFF = PAT1_OFF + 8 * NBIN               # [128 roi * 12]
WCOL1_OFF = WCOL0_OFF + NROI_CORE * 12        # [128 roi * 8]
CST_COLS = WCOL1_OFF + NROI_CORE * 8

# idx int16 column layout (per partition); all calls are per-ROI,
# trailing -1 entries are self-trimmed by the gather ucode (no DMA traffic).
# L2+L3 share one call: zones [0:384)=L2 (idx-0 padded), [384:512)=L3
# (idx offset by F2_ROWS into the concatenated f23 buffer, -1 tail).
IC0 = L0['NJ'] // 16                          # 25 cols per ROI
IC1 = L1['NJ'] // 16                          # 25
NJ23 = L2['NJ'] + L3['NJ']                    # 512
IC23 = NJ23 // 16                             # 32
IDX0_OFF = 0
IDX1_OFF = IDX0_OFF + NROI_CORE * IC0
IDX23_OFF = IDX1_OFF + NROI_CORE * IC1
IDX_COLS = IDX23_OFF + NROI_CORE * IC23

_MODULE_CACHE = {}


def _sample_meta(boxes_b, H, W, scale):
    """Per-ROI sample geometry in fp32, matching reference op order.
    boxes_b: [N, 4] fp32. Returns dict of [N,7,2] arrays."""
    f = np.float32
    b = boxes_b.astype(np.float32)
    x1 = b[:, 0] * f(scale)
    y1 = b[:, 1] * f(scale)
    x2 = b[:, 2] * f(scale)
    y2 = b[:, 3] * f(scale)
    rw = np.maximum(x2 - x1, f(1.0))
    rh = np.maximum(y2 - y1, f(1.0))
    bw = rw / f(POOLED)
    bh = rh / f(POOLED)
    g = (np.arange(POOLED, dtype=np.float32)[:, None]
         + (np.arange(SAMP, dtype=np.float32)[None, :] + f(0.5)) / f(SAMP))
    y = y1[:, None, None] + g[None] * bh[:, None, None]   # [N,7,2]
    x = x1[:, None, None] + g[None] * bw[:, None, None]
    masky = ((y >= f(-1.0)) & (y <= f(H))).astype(np.float32)
    maskx = ((x >= f(-1.0)) & (x <= f(W))).astype(np.float32)
    yc = np.clip(y, f(0.0), f(H - 1))
    xc = np.clip(x, f(0.0), f(W - 1))
    yl = np.floor(yc).astype(np.int64)
    xl = np.floor(xc).astype(np.int64)
    yh = np.minimum(yl + 1, H - 1)
    xh = np.minimum(xl + 1, W - 1)
    ly = (yc - yl.astype(np.float32)).astype(np.float32)
    lx = (xc - xl.astype(np.float32)).astype(np.float32)
    hy = (f(1.0) - ly).astype(np.float32)
    hx = (f(1.0) - lx).astype(np.float32)
    return dict(yl=yl, yh=yh, xl=xl, xh=xh, ly=ly, lx=lx, hy=hy, hx=hx,
                masky=masky, maskx=maskx, x=x, y=y)


def _build_tri(meta, lv):
    """L0: j = (rs, py, sy, px, sx) -> 392 3-px elems, 3 slot weights.
    Returns idx [N, NJ] int64, w [N, NJ, 3] fp32."""
    N = meta['yl'].shape[0]
    W = lv['W']
    NJ, REAL = lv['NJ'], lv['REAL']
    rows = np.stack([meta['yl'], meta['yh']], axis=1)          # [N,2,7,2] (rs)
    wys = np.stack([meta['hy'], meta['ly']], axis=1)           # [N,2,7,2]
    m = (meta['masky'][:, :, :, None, None] * meta['maskx'][:, None, None, :, :])  # [N,7,2,7,2]
    row = np.broadcast_to(rows[:, :, :, :, None, None], (N, 2, 7, 2, 7, 2))
    wy = np.broadcast_to(wys[:, :, :, :, None, None], (N, 2, 7, 2, 7, 2)).astype(np.float32)
    xl = np.broadcast_to(meta['xl'][:, None, None, None, :, :], (N, 2, 7, 2, 7, 2))
    hx = np.broadcast_to(meta['hx'][:, None, None, None, :, :], (N, 2, 7, 2, 7, 2)).astype(np.float32)
    lx = np.broadcast_to(meta['lx'][:, None, None, None, :, :], (N, 2, 7, 2, 7, 2)).astype(np.float32)
    mm = np.broadcast_to(m[:, None], (N, 2, 7, 2, 7, 2)).astype(np.float32)
    flat = row * W + xl
    idx = (flat >> 1).reshape(N, REAL)
    r = (flat & 1).astype(np.float32).reshape(N, REAL)
    wl = (wy * hx * mm * np.float32(0.25)).reshape(N, REAL)
    wh = (wy * lx * mm * np.float32(0.25)).reshape(N, REAL)
    w = np.zeros((N, NJ, 3), np.float32)
    w[:, :REAL, 0] = wl * (1 - r)
    w[:, :REAL, 1] = wl * r + wh * (1 - r)
    w[:, :REAL, 2] = wh * r
    idx_full = np.full((N, NJ), -1, np.int64)
    idx_full[:, :REAL] = idx
    return idx_full, w


def _build_pair(meta, lv):
    """2px overlapping elems at 1px step: elem j=(rs,py,sy,px,sx) starts at
    (row, xl): slot0=xl (wy*hx), slot1=xl+1 (wy*lx). idx = row*W + xl."""
    N = meta['yl'].shape[0]
    W = lv['W']
    NJ, REAL = lv['NJ'], lv['REAL']
    rows = np.stack([meta['yl'], meta['yh']], axis=1)          # [N,2,7,2]
    wys = np.stack([meta['hy'], meta['ly']], axis=1)
    m = (meta['masky'][:, :, :, None, None] * meta['maskx'][:, None, None, :, :])
    row = np.broadcast_to(rows[:, :, :, :, None, None], (N, 2, 7, 2, 7, 2))
    wy = np.broadcast_to(wys[:, :, :, :, None, None], (N, 2, 7, 2, 7, 2)).astype(np.float32)
    xl = np.broadcast_to(meta['xl'][:, None, None, None, :, :], (N, 2, 7, 2, 7, 2))
    hx = np.broadcast_to(meta['hx'][:, None, None, None, :, :], (N, 2, 7, 2, 7, 2)).astype(np.float32)
    lx = np.broadcast_to(meta['lx'][:, None, None, None, :, :], (N, 2, 7, 2, 7, 2)).astype(np.float32)
    mm = np.broadcast_to(m[:, None], (N, 2, 7, 2, 7, 2)).astype(np.float32)
    idx = (row * W + xl).reshape(N, REAL)
    w = np.zeros((N, NJ, 2), np.float32)
    w[:, :REAL, 0] = (wy * hx * mm * np.float32(0.25)).reshape(N, REAL)
    w[:, :REAL, 1] = (wy * lx * mm * np.float32(0.25)).reshape(N, REAL)
    idx_full = np.full((N, NJ), -1, np.int64)
    idx_full[:, :REAL] = idx
    return idx_full, w


def _build_px(meta, lv):
    """L1: j = (rs, cs, py, sy, px, sx) -> 784 1-px corner gathers.
    Returns idx [N, NJ] int64, w [N, NJ] fp32."""
    N = meta['yl'].shape[0]
    W = lv['W']
    NJ, REAL = lv['NJ'], lv['REAL']
    rows = np.stack([meta['yl'], meta['yh']], axis=1)   # [N,2(rs),7,2]
    wys = np.stack([meta['hy'], meta['ly']], axis=1)
    cols = np.stack([meta['xl'], meta['xh']], axis=1)   # [N,2(cs),7,2]
    wxs = np.stack([meta['hx'], meta['lx']], axis=1)
    m = (meta['masky'][:, :, :, None, None] * meta['maskx'][:, None, None, :, :])
    row = np.broadcast_to(rows[:, :, None, :, :, None, None], (N, 2, 2, 7, 2, 7, 2))
    wy = np.broadcast_to(wys[:, :, None, :, :, None, None], (N, 2, 2, 7, 2, 7, 2)).astype(np.float32)
    col = np.broadcast_to(cols[:, None, :, None, None, :, :], (N, 2, 2, 7, 2, 7, 2))
    wx = np.broadcast_to(wxs[:, None, :, None, None, :, :], (N, 2, 2, 7, 2, 7, 2)).astype(np.float32)
    mm = np.broadcast_to(m[:, None, None], (N, 2, 2, 7, 2, 7, 2)).astype(np.float32)
    idx = (row * W + col).reshape(N, REAL)
    w = (wy * wx * mm * np.float32(0.25)).reshape(N, REAL)
    idx_full = np.full((N, NJ), -1, np.int64)
    w_full = np.zeros((N, NJ), np.float32)
    idx_full[:, :REAL] = idx
    w_full[:, :REAL] = w
    return idx_full, w_full


def _build_reg(meta, lv, pad_value=0):
    """L2/L3: bounding-region pixels + separable host-baked weights.
    Returns idx [N, NJ] int64, lhsT [N, NJ, 49] fp32.
    pad_value=-1 trims pad descriptors (only safe when NJ is one 128-chunk)."""
    N = meta['yl'].shape[0]
    H, W, WREG = lv['H'], lv['W'], lv['WREG']
    NJ, REAL = lv['NJ'], lv['REAL']
    f = np.float32
    y_base = np.floor(np.clip(meta['y'].reshape(N, -1).min(1), 0.0, H - 1)).astype(np.int64)
    x_base = np.floor(np.clip(meta['x'].reshape(N, -1).min(1), 0.0, W - 1)).astype(np.int64)
    WY = np.zeros((N, WREG, POOLED), np.float32)
    WX = np.zeros((N, WREG, POOLED), np.float32)
    ridx = np.arange(N)[:, None, None]
    pidx = np.broadcast_to(np.arange(POOLED)[None, :, None], (N, POOLED, SAMP))
    np.add.at(WY, (ridx, meta['yl'] - y_base[:, None, None], pidx),
              (f(0.5) * meta['hy'] * meta['masky']).astype(np.float32))
    np.add.at(WY, (ridx, meta['yh'] - y_base[:, None, None], pidx),
              (f(0.5) * meta['ly'] * meta['masky']).astype(np.float32))
    np.add.at(WX, (ridx, meta['xl'] - x_base[:, None, None], pidx),
              (f(0.5) * meta['hx'] * meta['maskx']).astype(np.float32))
    np.add.at(WX, (ridx, meta['xh'] - x_base[:, None, None], pidx),
              (f(0.5) * meta['lx'] * meta['maskx']).astype(np.float32))
    # variable spans: last nonzero WY/WX row per ROI -> sy, sx
    ny = (WY.reshape(N, WREG, -1) != 0).any(-1)    # [N, WREG]
    nx = (WX.reshape(N, WREG, -1) != 0).any(-1)
    sy = np.maximum(WREG - np.argmax(ny[:, ::-1], axis=1), 1)
    sx = np.maximum(WREG - np.argmax(nx[:, ::-1], axis=1), 1)
    lhsT_grid = np.einsum('nap,nbq->nabpq', WY, WX).reshape(N, WREG, WREG, NBIN)
    idx_full = np.full((N, NJ), pad_value, np.int64)
    lhsT_full = np.zeros((N, NJ, NBIN), np.float32)
    for n in range(N):
        yy, xx = int(sy[n]), int(sx[n])
        cnt = yy * xx
        dy = np.arange(yy)
        dx = np.arange(xx)
        idx_full[n, :cnt] = ((y_base[n] + dy[:, None]) * W
                             + x_base[n] + dx[None, :]).reshape(-1)
        lhsT_full[n, :cnt] = lhsT_grid[n, :yy, :xx].reshape(cnt, NBIN)
    return idx_full, lhsT_full


def _pack_idx(jlists):
    """Pack concatenated idx list [..., n] -> [..., 128, n//16]
    int16 wrapped in 16 partitions, replicated 8x."""
    jl = np.asarray(jlists)
    n = jl.shape[-1]
    arr = jl.reshape(*jl.shape[:-1], n // 16, 16)   # [..., col, p]
    arr = np.swapaxes(arr, -1, -2)                  # [..., p(16), col]
    arr = np.broadcast_to(arr[..., None, :, :],
                          (*jl.shape[:-1], 8, 16, n // 16))
    return arr.reshape(*jl.shape[:-1], 128, n // 16).astype(np.int16)


def _bin_pattern(mode, NCH, REAL, nslot):
    """Fixed j->bin one-hot pattern [128, NCH*nslot, 49] (expanded per slot)."""
    NJ = NCH * 128
    j = np.arange(NJ)
    # j = ((((rs*7+py)*2+sy)*7+px)*2+sx)  (same py/px decode for tri & px)
    px = (j // 2) % 7
    py = (j // (2 * 7 * 2)) % 7
    bins = py * 7 + px
    pat = np.zeros((NJ, NBIN), np.float32)
    valid = j < REAL
    pat[np.arange(NJ)[valid], bins[valid]] = 1.0
    pat = pat.reshape(NCH, 128, NBIN).transpose(1, 0, 2)          # [128, NCH, 49]
    pat = np.repeat(pat[:, :, None, :], nslot, axis=2)            # [128, NCH, nslot, 49]
    return pat.reshape(128, NCH * nslot, NBIN)


def _host_prepare(x0, x1, x2, x3, boxes):
    """Build all per-core input tensors. Returns list of 8 dicts."""
    B = boxes.shape[0]
    feats = []
    for arr, lv, rows in ((x0, L0, F0_ROWS), (x1, L1, F1_ROWS),
                          (x2, L2, F2_ROWS), (x3, L3, F3_ROWS)):
        f = np.zeros((B, rows, C), BF16)
        hw = lv['H'] * lv['W']
        f[:, :hw] = np.ascontiguousarray(
            np.transpose(np.asarray(arr, np.float32), (0, 2, 3, 1))).reshape(B, hw, C).astype(BF16)
        feats.append(f)
    f23 = np.concatenate([feats[2], feats[3]], axis=1)   # [B, F2+F3 rows, C]

    per_batch = []
    for b in range(B):
        bb = np.asarray(boxes[b], np.float32)
        m0 = _sample_meta(bb, L0['H'], L0['W'], L0['scale'])
        m1 = _sample_meta(bb, L1['H'], L1['W'], L1['scale'])
        m2 = _sample_meta(bb, L2['H'], L2['W'], L2['scale'])
        m3 = _sample_meta(bb, L3['H'], L3['W'], L3['scale'])
        idx0, w0 = _build_tri(m0, L0)
        idx1, w1 = _build_pair(m1, L1)
        idx2, lt2 = _build_reg(m2, L2, pad_value=0)
        idx3, lt3 = _build_reg(m3, L3, pad_value=-1)
        per_batch.append((idx0, w0, idx1, w1, idx2, lt2, idx3, lt3))

    pat0 = _bin_pattern('tri', L0['NCH'], L0['REAL'], 3)   # [128, 12, 49]
    pat1 = _bin_pattern('pair', L1['NCH'], L1['REAL'], 2)  # [128, 8, 49]

    in_maps = []
    for k in range(8):
        b = k // 4
        s = (k % 4) * NROI_CORE
        idx0, w0, idx1, w1, idx2, lt2, idx3, lt3 = per_batch[b]
        sl = slice(s, s + NROI_CORE)

        cst = np.zeros((128, CST_COLS), BF16)
        cst[:, PAT0_OFF:PAT0_OFF + 12 * NBIN] = pat0.reshape(128, -1).astype(BF16)
        cst[:, PAT1_OFF:PAT1_OFF + 8 * NBIN] = pat1.reshape(128, -1).astype(BF16)
        # wcol0 [128, roi*12]: col roi*12 + c*3 + slot = w0[roi, c*128+p, slot]
        w0p = np.zeros((NROI_CORE, L0['NCH'] * 128, 3), np.float32)
        w0p[:, :L0['NJ']] = w0[sl]
        wc0 = w0p.reshape(NROI_CORE, L0['NCH'], 128, 3)      # [roi,c,p,s]
        cst[:, WCOL0_OFF:WCOL0_OFF + NROI_CORE * 12] = (
            wc0.transpose(2, 0, 1, 3).reshape(128, -1).astype(BF16))
        w1p = np.zeros((NROI_CORE, L1['NCH'] * 128, 2), np.float32)
        w1p[:, :L1['NJ']] = w1[sl]
        wc1 = w1p.reshape(NROI_CORE, L1['NCH'], 128, 2)      # [roi,c,p,s]
        cst[:, WCOL1_OFF:WCOL1_OFF + NROI_CORE * 8] = (
            wc1.transpose(2, 0, 1, 2 + 1).reshape(128, -1).astype(BF16))

        idx3s = idx3[sl].copy()
        idx3s[idx3s >= 0] += F2_ROWS
        idx23 = np.concatenate([idx2[sl], idx3s], axis=1)    # [128roi, 512]
        idxs = np.zeros((128, IDX_COLS), np.int16)
        idxs[:, IDX0_OFF:IDX0_OFF + NROI_CORE * IC0] = _pack_idx(
            idx0[sl]).transpose(1, 0, 2).reshape(128, -1)
        idxs[:, IDX1_OFF:IDX1_OFF + NROI_CORE * IC1] = _pack_idx(
            idx1[sl]).transpose(1, 0, 2).reshape(128, -1)
        idxs[:, IDX23_OFF:IDX23_OFF + NROI_CORE * IC23] = _pack_idx(
            idx23).transpose(1, 0, 2).reshape(128, -1)

        # lhsT k-major bf16: lt2 [roi, NJ(=3*128), 49] -> [roi, 128, 3*49]
        lt2k = np.ascontiguousarray(
            lt2[sl].reshape(NROI_CORE, L2['NCH'], 128, NBIN).transpose(0, 2, 1, 3)
        ).reshape(NROI_CORE, 128, L2['NCH'] * NBIN).astype(BF16)
        lt3k = np.ascontiguousarray(lt3[sl].reshape(NROI_CORE, 128, NBIN)).astype(BF16)

        in_maps.append({
            "f0": feats[0][b], "f1": feats[1][b], "f23": f23[b],
            "cst": cst, "idxs": idxs, "lt2": lt2k, "lt3": lt3k,
        })
    return in_maps


def _build_module():
    from concourse import bacc, tile
    from concourse.bass import mybir
    import concourse.bass as bass_mod

    F32 = mybir.dt.float32
    BF = mybir.dt.bfloat16
    I16 = mybir.dt.int16
    AP = bass_mod.AP

    nc = bacc.Bacc(None, target_bir_lowering=False, num_swdge_queues=4)
    f0 = nc.dram_tensor("f0", [F0_ROWS, C], BF, kind="ExternalInput")
    f1 = nc.dram_tensor("f1", [F1_ROWS, C], BF, kind="ExternalInput")
    f23 = nc.dram_tensor("f23", [F2_ROWS + F3_ROWS, C], BF, kind="ExternalInput")
    cst = nc.dram_tensor("cst", [128, CST_COLS], BF, kind="ExternalInput")
    idxs = nc.dram_tensor("idxs", [128, IDX_COLS], I16, kind="ExternalInput")
    lt2 = nc.dram_tensor("lt2", [NROI_CORE, 128, L2['NCH'] * NBIN], BF, kind="ExternalInput")
    lt3 = nc.dram_tensor("lt3", [NROI_CORE, 128, NBIN], BF, kind="ExternalInput")
    out = nc.dram_tensor("out", [NROI_CORE, NBIN, C], BF, kind="ExternalOutput")

    # overlapping 3-px elem views: stride 2px, width 3px
    f0_view = AP(f0, 0, [[2 * C, F0_ROWS // 2 - 1], [1, 3 * C]])
    f1_view = AP(f1, 0, [[C, F1_ROWS - 1], [1, 2 * C]])

    with tile.TileContext(nc) as tc:
        with (
            tc.tile_pool(name="const", bufs=1) as constp,
            tc.tile_pool(name="g0p", bufs=5) as g0p,
            tc.tile_pool(name="g1p", bufs=5) as g1p,
            tc.tile_pool(name="g2p", bufs=6) as g2p,
            tc.tile_pool(name="ltp", bufs=4) as ltp,
            tc.tile_pool(name="wp", bufs=8) as wp,
            tc.tile_pool(name="accp", bufs=8, space="PSUM") as accp,
            tc.tile_pool(name="evp", bufs=3) as evp,
        ):
            cst_t = constp.tile([128, CST_COLS], BF)
            nc.sync.dma_start(cst_t[:], cst[:])
            idx_t = constp.tile([128, IDX_COLS], I16)
            nc.sync.dma_start(idx_t[:], idxs[:])

            pat0_ap = cst_t[:, PAT0_OFF:PAT0_OFF + 12 * NBIN].rearrange(
                "p (c b) -> p c b", b=NBIN)
            pat1_ap = cst_t[:, PAT1_OFF:PAT1_OFF + 8 * NBIN].rearrange(
                "p (c b) -> p c b", b=NBIN)
            wcol0_ap = cst_t[:, WCOL0_OFF:WCOL0_OFF + NROI_CORE * 12].rearrange(
                "p (r c) -> p r c", c=12)
            wcol1_ap = cst_t[:, WCOL1_OFF:WCOL1_OFF + NROI_CORE * 8].rearrange(
                "p (r c) -> p r c", c=8)

            # zero the rotating gather buffers once: trimmed gather calls leave
            # pad rows unwritten, and the zero-weight protection needs finite
            # data there (never-NaN) on the very first uses.
            for pool, shape, tg, nb in ((g0p, [128, GRP * L0['NCH'], 3 * C], "g0", 5),
                                        (g1p, [128, GRP * L1['NCH'], 2 * C], "g1", 5),
                                        (g2p, [128, GRP * 4, C], "g2", 6)):
                for _ in range(nb):
                    t = pool.tile(shape, BF, tag=tg)
                    nc.vector.memset(t[:], 0)

            for grp in range(NGRP):
                # per-ROI gathers, all four levels, rotating queues
                gt0 = g0p.tile([128, GRP * L0['NCH'], 3 * C], BF, tag="g0")
                gt1 = g1p.tile([128, GRP * L1['NCH'], 2 * C], BF, tag="g1")
                gt23 = g2p.tile([128, GRP * 4, C], BF, tag="g2")
                for r2 in range(GRP):
                    roi = grp * GRP + r2
                    io = IDX0_OFF + roi * IC0
                    nc.gpsimd.dma_gather(
                        gt0[:, r2 * L0['NCH']:(r2 + 1) * L0['NCH'], :], f0_view,
                        idx_t[:, io:io + IC0],
                        L0['NJ'], L0['REAL'], 3 * C, elem_step=2 * C,
                        queue_num=roi % 4)
                    io = IDX1_OFF + roi * IC1
                    nc.gpsimd.dma_gather(
                        gt1[:, r2 * L1['NCH']:(r2 + 1) * L1['NCH'], :], f1_view,
                        idx_t[:, io:io + IC1],
                        L1['NJ'], L1['REAL'], 2 * C, elem_step=C,
                        queue_num=(roi + 1) % 4)
                    io = IDX23_OFF + roi * IC23
                    nc.gpsimd.dma_gather(
                        gt23[:, r2 * 4:(r2 + 1) * 4, :], f23[:],
                        idx_t[:, io:io + IC23],
                        NJ23, NJ23, C, queue_num=(roi + 2) % 4)
                if grp % 4 == 0:
                    lt2_t = ltp.tile([128, 4 * GRP, L2['NCH'] * NBIN], BF, tag="lt2")
                    nc.sync.dma_start(
                        lt2_t[:],
                        lt2[grp * GRP:(grp + 4) * GRP].rearrange("r p m -> p r m"))
                    lt3_t = ltp.tile([128, 4 * GRP, NBIN], BF, tag="lt3")
                    nc.sync.dma_start(
                        lt3_t[:],
                        lt3[grp * GRP:(grp + 4) * GRP].rearrange("r p m -> p r m"))

                if grp % 2 == 0:
                    ev = evp.tile([NBIN, 2 * GRP, C], BF, tag="ev")
                for r2 in range(GRP):
                    roi = grp * GRP + r2
                    # build W tiles: pattern * per-partition scalar (broadcast)
                    w0 = wp.tile([128, 12, NBIN], BF, tag="w0")
                    nc.vector.tensor_tensor(
                        w0[:], pat0_ap,
                        wcol0_ap[:, roi, :].unsqueeze(2).to_broadcast([128, 12, NBIN]),
                        mybir.AluOpType.mult)
                    w1 = wp.tile([128, 8, NBIN], BF, tag="w1")
                    nc.vector.tensor_tensor(
                        w1[:], pat1_ap,
                        wcol1_ap[:, roi, :].unsqueeze(2).to_broadcast([128, 8, NBIN]),
                        mybir.AluOpType.mult)

                    acc = accp.tile([NBIN, C], F32)
                    n_mm = 12 + 8 + 3 + 1
                    mi = 0
                    for gt, w_t, lv, nslot in ((gt0, w0, L0, 3), (gt1, w1, L1, 2)):
                        for c in range(lv['NCH']):
                            for sl3 in range(nslot):
                                nc.tensor.matmul(
                                    acc[:], w_t[:, c * nslot + sl3, :],
                                    gt[:, r2 * lv['NCH'] + c, sl3 * C:(sl3 + 1) * C],
                                    start=(mi == 0), stop=(mi == n_mm - 1))
                                mi += 1
                    lt2_ap = lt2_t[:, (grp % 4) * GRP + r2, :].rearrange(
                        "p (c b) -> p c b", b=NBIN)
                    for c in range(L2['NCH']):
                        nc.tensor.matmul(
                            acc[:], lt2_ap[:, c, :],
                            gt23[:, r2 * 4 + c, :],
                            start=(mi == 0), stop=(mi == n_mm - 1))
                        mi += 1
                    nc.tensor.matmul(
                        acc[:], lt3_t[:, (grp % 4) * GRP + r2, :],
                        gt23[:, r2 * 4 + 3, :],
                        start=(mi == 0), stop=(mi == n_mm - 1))
                    mi += 1

                    nc.scalar.copy(ev[:, (grp % 2) * GRP + r2, :], acc[:])

                if grp % 2 == 1:
                    dst = out[(grp - 1) * GRP:(grp + 1) * GRP].rearrange(
                        "r b c -> b r c")
                    nc.sync.dma_start(dst, ev[:])
    nc.finalize()
    return nc


def kernel(x0, x1, x2, x3, boxes):
    from concourse.bass_utils import run_bass_kernel_spmd
    in_maps = _host_prepare(x0, x1, x2, x3, boxes)
    if 'nc' not in _MODULE_CACHE:
        _MODULE_CACHE['nc'] = _build_module()
    nc = _MODULE_CACHE['nc']
    res = run_bass_kernel_spmd(nc, in_maps, list(range(8)))
    globals()['_LAST_RESULTS'] = res
    outs = [np.asarray(res.results[k]["out"]) for k in range(8)]
    full = np.concatenate(outs, axis=0)            # [1024, 49, 256] bf16
    full = full.astype(np.float32).transpose(0, 2, 1)
    return np.ascontiguousarray(full.reshape(1024, C, POOLED, POOLED))
